# revision 1
# baseline (speedup 1.0000x reference)
"""Trainium2 Bass kernel: gated MoE residual block (two 3x3 convs, C=32).

  g  = gate * (gate > 0)                          # [B, C]
  h  = relu((conv3x3(x, w1) + b1) * g)
  h2 = relu((conv3x3(h, w2) + b2) * g)
  out = h2 + x

Sharding: data-parallel over batch. 16 images -> 8 cores x 2 images.

Device algorithm (per core, per image):
  - x arrives pre-packed (host-side numpy) in "mod-4 row-interleaved" SBUF
    layout: partition 32*(row%4)+ci, free = (row//4, col), zero halo baked
    in. A second copy arrives pre-rotated by 2 rows for the residual add.
    All device DMAs are fully contiguous (128 long descriptors each).
  - conv as full-size matmuls, K = M = 128: contraction over 4 row-slots x
    32 channels of one aligned 4-row window; output columns (q, co) hold 4
    CONSECUTIVE output rows (window rows + 1). Each output row's 3 dy-taps
    split between the aligned window (main) and the next window (wrap):
    2 matmuls per dx, 6 per 8-row PSUM block, all base-partition 0.
  - h stays on-chip with +1 row phase so conv2 reuses the same structure.
  - epilogue on ScalarE: relu(psum * g + b*g) straight from PSUM.
  - conv2 epilogue + residual add on VectorE into a full-image staging
    buffer, stored with one contiguous DMA; host de-interleaves.
"""

import numpy as np
import ml_dtypes

import concourse.bass as bass
import concourse.tile as tile
from concourse import bacc, mybir

B, C, H, W = 16, 32, 256, 256
IMGS_PER_CORE = 2
N_CORES = 8
KW = 3
S = 4            # row interleave factor (slots per window)
A = H // S       # 64 aligned 4-row windows
WP = W + 2       # padded row width (zero cols 0 and 257)
NSX = A + 3      # x_il slots: idx = window + 1; idx 0, A+1, A+2 zero
NSR = A + 2      # x_rot/out_stage slots (phase-2): idx 0..A+1
J = 2            # windows per PSUM block: N = J*W = 512
F32 = mybir.dt.float32
BF16 = mybir.dt.bfloat16
NV = 2 * KW      # conv2 weight matrices: (main, wrap) x 3 dx
NV1 = KW + 2     # conv1: 3 mains + 2 packed wraps (dx folded into K-slots)
BLOCKS = [-1] + list(range(1, A, J))


def _pack_weights(w: np.ndarray) -> np.ndarray:
    """w: [C_out, C_in, 3, 3] (OIHW) -> [NV, 128, 128] lhsT stack.

    Block (s, q) of main[dx] = w[:, :, s-q, dx].T   (0 <= s-q <= 2)
    Block (s, q) of wrap[dx] = w[:, :, 4+s-q, dx].T (0 <= 4+s-q <= 2)
    lhsT[(32s+ci), (32q+co)]; out row (window k) = 4k+1+q.
    """
    wv = np.zeros((NV, S * C, S * C), dtype=np.float32)
    for dx in range(KW):
        for q in range(S):
            for s in range(S):
                if 0 <= s - q <= 2:
                    wv[2 * dx, 32 * s:32 * s + 32, 32 * q:32 * q + 32] = \
                        w[:, :, s - q, dx].T
                if 0 <= 4 + s - q <= 2:
                    wv[2 * dx + 1, 32 * s:32 * s + 32, 32 * q:32 * q + 32] = \
                        w[:, :, 4 + s - q, dx].T
    return wv


def _pack_weights_wrapped(w: np.ndarray) -> np.ndarray:
    """conv1 weights: [NV1, 128, 128] = 3 mains (as _pack_weights) + 2
    packed wraps reading x_wrap (partition e=2c+r; c = dx-copy, r = row).

    wrapA (u offset 0): cell (e=2c+r, q) covers dx=c taps;
    wrapB (u offset 1): cells with c=1 cover dx=2.
    taps: q=2 <- (r0, dy2); q=3 <- (r0, dy1), (r1, dy2).
    """
    full = _pack_weights(w)
    wv = np.zeros((NV1, S * C, S * C), dtype=np.float32)
    for dx in range(KW):
        wv[dx] = full[2 * dx]          # mains
    taps = [(2, 0, 2), (3, 0, 1), (3, 1, 2)]   # (q, r, dy)
    for q, r, dy in taps:
        for c in (0, 1):
            e = 2 * c + r
            wv[KW, 32 * e:32 * e + 32, 32 * q:32 * q + 32] = w[:, :, dy, c].T
        e = 2 * 1 + r
        wv[KW + 1, 32 * e:32 * e + 32, 32 * q:32 * q + 32] = w[:, :, dy, 2].T
    return wv


def _wrap_x(x: np.ndarray) -> np.ndarray:
    """x: [n, C, H, W] -> x_wrap [n, 128, A, WP] bf16.

    partition 32*(2c+r)+ci, slot t, col u = x[ci, 4t+r, u-1+c] (zero pad).
    """
    n = x.shape[0]
    xb = x.astype(ml_dtypes.bfloat16)
    out = np.zeros((n, S * C, A, WP), dtype=ml_dtypes.bfloat16)
    for c in (0, 1):
        for r in (0, 1):
            e = 2 * c + r
            rows = xb[:, :, r::S, :]               # [n, C, A, W]
            out[:, 32 * e:32 * e + 32, :, 1 - c:1 - c + W] = rows
    return np.ascontiguousarray(out)


def _interleave_x(x: np.ndarray) -> np.ndarray:
    """x: [n, C, H, W] f32 -> x_il [n,128,NSX,WP] bf16.

    x_il: partition 32s+ci holds row 4(i-1)+s at slot i, col c+1 (zero halo).
    """
    n = x.shape[0]
    xb = x.astype(ml_dtypes.bfloat16)

    ext = np.zeros((n, C, S * NSX, W), dtype=ml_dtypes.bfloat16)
    ext[:, :, S:S + H, :] = xb
    il = ext.reshape(n, C, NSX, S, W).transpose(0, 3, 1, 2, 4) \
            .reshape(n, S * C, NSX, W)
    x_il = np.zeros((n, S * C, NSX, WP), dtype=ml_dtypes.bfloat16)
    x_il[:, :, :, 1:1 + W] = il

    return np.ascontiguousarray(x_il)


def _deinterleave_out(dev: np.ndarray) -> np.ndarray:
    """dev: [n, 128, NSR, W] (row z = 4(i-1)+2+q at partition 32q+co)
    -> [n, C, H, W] f32."""
    dev = np.asarray(dev).astype(np.float32)
    n = dev.shape[0]
    v = dev.reshape(n, S, C, NSR, W).transpose(0, 2, 3, 1, 4) \
           .reshape(n, C, S * NSR, W)
    return np.ascontiguousarray(v[:, :, 2:2 + H, :])


def _build_core_graph(reps: int = 1):
    nc = bacc.Bacc(None, target_bir_lowering=False, debug=False)

    xil_ext = nc.declare_dram_parameter("xil", [IMGS_PER_CORE, S * C, NSX, WP], BF16, isOutput=False)
    wv1_ext = nc.declare_dram_parameter("wv1", [S * C, NV1, S * C], BF16, isOutput=False)
    xw_ext = nc.declare_dram_parameter("xw", [IMGS_PER_CORE, S * C, A, WP], BF16, isOutput=False)
    wv2_ext = nc.declare_dram_parameter("wv2", [S * C, NV1, S * C], BF16, isOutput=False)
    gv_ext = nc.declare_dram_parameter("gv", [S * C, IMGS_PER_CORE], F32, isOutput=False)
    bg1_ext = nc.declare_dram_parameter("bg1", [S * C, IMGS_PER_CORE], F32, isOutput=False)
    bg2_ext = nc.declare_dram_parameter("bg2", [S * C, IMGS_PER_CORE], F32, isOutput=False)
    out_ext = nc.declare_dram_parameter("out", [IMGS_PER_CORE, S * C, NSR, W], BF16, isOutput=True)

    with tile.TileContext(nc) as tc:
        with (
            tc.tile_pool(name="const", bufs=1) as cpool,
            tc.tile_pool(name="xb", bufs=1) as xpool,
            tc.tile_pool(name="os", bufs=1) as ospool,
            tc.tile_pool(name="hb", bufs=1) as hpool,
            tc.tile_pool(name="ps", bufs=8, space=bass.MemorySpace.PSUM) as pspool,
            tc.tile_pool(name="ep", bufs=4) as epool,
        ):
            wv1_t = cpool.tile([S * C, NV1, S * C], BF16)
            wv2_t = cpool.tile([S * C, NV1, S * C], BF16)
            gv_t = cpool.tile([S * C, IMGS_PER_CORE], F32)
            bg1_t = cpool.tile([S * C, IMGS_PER_CORE], F32)
            bg2_t = cpool.tile([S * C, IMGS_PER_CORE], F32)
            # constants issue from otherwise-idle engines so SP can start
            # streaming x immediately (SP DMA issue is serial, ~1us each)
            # first-needed weights (block -1's wraps) go at the head of
            # SP's queue; ACT is blocked by its activation-table load early
            # PE warm-up: dummy matmuls on zeroed scratch start the clock
            # ramp before the first real operands arrive (results unread)
            warm = cpool.tile([S * C, W], BF16, tag="warm")
            nc.vector.memset(warm[:], 0.0)
            wps = pspool.tile([S * C, J, W], F32, tag="ps")
            for wi_ in range(5):
                nc.tensor.matmul(
                    wps[:, 0, :], warm[:, 0:S * C], warm[:],
                    start=True, stop=True, skip_group_check=True)

            nc.sync.dma_start(out=wv1_t[:, KW:, :], in_=wv1_ext[:, KW:, :])
            nc.scalar.dma_start(out=wv1_t[:, 0:KW, :], in_=wv1_ext[:, 0:KW, :])
            nc.scalar.dma_start(out=wv2_t[:], in_=wv2_ext[:])

            for img in [i for _ in range(reps) for i in range(IMGS_PER_CORE)]:
                x_il = xpool.tile([S * C, NSX, WP], BF16)
                x_wrap = xpool.tile([S * C, A, WP], BF16, tag="x_wrap")
                h_wrap = xpool.tile([S * C, A, WP], BF16, tag="h_wrap")
                out_stage = ospool.tile([S * C, NSR, W], BF16)
                h_il = hpool.tile([S * C, NSX, WP], BF16)

                # interleave x_il / x_wrap chunk issue by first-need order
                # (SP issues DMAs serially; block k0 needs x_il idx <= k0+3
                # and x_wrap slot <= k0+2)
                # first x_wrap chunk issues from Pool so it lands in
                # parallel with SP's first x_il chunk
                nc.gpsimd.dma_start(out=x_wrap[:, 0:3, :],
                                    in_=xw_ext[img, :, 0:3, :])
                if img == 0:
                    # small consts are only needed at the first epilogue
                    nc.gpsimd.dma_start(out=gv_t[:], in_=gv_ext[:])
                    nc.gpsimd.dma_start(out=bg1_t[:], in_=bg1_ext[:])
                    nc.gpsimd.dma_start(out=bg2_t[:], in_=bg2_ext[:])
                for which, c0, c1 in (
                    ("il", 0, 4), ("il", 4, 9), ("w", 3, 8),
                    ("il", 9, 17), ("w", 8, 16), ("il", 17, 33),
                    ("w", 16, 32), ("il", 33, 50), ("w", 32, A),
                    ("il", 50, NSX),
                ):
                    if which == "il":
                        nc.sync.dma_start(out=x_il[:, c0:c1, :],
                                          in_=xil_ext[img, :, c0:c1, :])
                    else:
                        nc.sync.dma_start(out=x_wrap[:, c0:c1, :],
                                          in_=xw_ext[img, :, c0:c1, :])

                # h halo: zero slots 0, A+1, A+2 and cols 0, WP-1
                nc.vector.memset(h_il[:, 0, :], 0.0)
                nc.vector.memset(h_il[3 * C:4 * C, A, :], 0.0)
                nc.vector.memset(h_il[:, A + 1, :], 0.0)
                nc.vector.memset(h_il[:, A + 2, :], 0.0)
                nc.vector.memset(h_il[:, :, 0], 0.0)
                nc.vector.memset(h_il[:, :, WP - 1], 0.0)

                def issue_group(mms, jn):
                    ps = pspool.tile([S * C, J, W], F32, tag="ps")
                    for n, (lhs, rhs) in enumerate(mms):
                        nc.tensor.matmul(
                            ps[:, 0:jn, :], lhs, rhs,
                            start=(n == 0), stop=(n == len(mms) - 1),
                            skip_group_check=True,
                        )
                    return ps

                def conv_blocks(src, wv_t, wrap_src, first_main_is_pad,
                                order=BLOCKS):
                    mains = lambda k0, lo, hi: [
                        (wv_t[:, dx, :], src[:, lo:hi, dx:dx + W])
                        for dx in range(KW)]
                    wraps = lambda lo, hi: [
                        (wv_t[:, KW + wb, :], wrap_src[:, lo:hi, wb:wb + W])
                        for wb in (0, 1)]
                    for k0 in order:
                        if k0 == -1 and first_main_is_pad:
                            # conv1 only: the j=0 main window is all x-pad,
                            # so split into two uniform N=256 groups
                            yield k0, issue_group(wraps(0, 1), 1), 0, 1
                            yield k0, issue_group(
                                mains(k0, 1, 2) + wraps(1, 2), 1), 1, 1
                        elif k0 == A - 1:
                            # no wraps; j=1 window is all padding
                            yield k0, issue_group(mains(k0, A, A + 1), 1), 0, 1
                        else:
                            yield k0, issue_group(
                                mains(k0, k0 + 1, k0 + 1 + J)
                                + wraps(k0 + 1, k0 + 1 + J), J), 0, J

                # ---- conv1: x_il -> h_il (h stored with +1 row phase) ----
                # edge blocks write only their valid rows so the h halo
                # (zeroed once above) is never dirtied
                for k0, ps, j0, jn in conv_blocks(x_il, wv1_t, x_wrap, True):
                    RELU = mybir.ActivationFunctionType.Relu

                    def ep1(p0, p1, hs, js):
                        nc.scalar.activation(
                            h_il[p0:p1, hs, 1:1 + W], ps[p0:p1, js, :], RELU,
                            bias=bg1_t[p0:p1, img:img + 1],
                            scale=gv_t[p0:p1, img:img + 1])

                    if k0 == -1 and j0 == 0:
                        # only row 0 (q=3) is a real output of this group
                        ep1(3 * C, 4 * C, slice(0, 1), slice(0, 1))
                    elif k0 == A - 1:
                        ep1(0, 3 * C, slice(A, A + 1), slice(0, 1))
                    else:
                        ep1(0, 4 * C,
                            slice(k0 + 1 + j0, k0 + 1 + j0 + jn),
                            slice(0, jn))

                    # h_wrap chunks: [t0:t1] needs h_il idx up to t1 which is
                    # complete once block k0 = t1-1 has written idx t1
                    hw_chunks = {31: (0, 32), 63: (32, A)}
                    if k0 in hw_chunks:
                        t0, t1 = hw_chunks[k0]
                        for r in (0, 1):
                            # c=0 copy (contiguous): h_wrap u <- h_il col u
                            eng0 = nc.sync if r == 0 else nc.gpsimd
                            eng0.dma_start(
                                out=h_wrap[32 * r:32 * r + 32, t0:t1, :],
                                in_=h_il[32 * r:32 * r + 32,
                                         1 + t0:1 + t1, :],
                            )
                            # c=1 copy (1-col shift): u <- h_il col u+1
                            eng1 = nc.gpsimd if r == 0 else nc.sync
                            eng1.dma_start(
                                out=h_wrap[64 + 32 * r:96 + 32 * r,
                                           t0:t1, 0:WP - 1],
                                in_=h_il[32 * r:32 * r + 32,
                                         1 + t0:1 + t1, 1:WP],
                            )

                # ---- conv2 + residual into out_stage ----
                for m0, ps, j0, jn in conv_blocks(h_il, wv2_t, h_wrap, False):
                    # h2 = relu(conv2*g + b*g) straight into the staging
                    # buffer; the residual +x happens host-side in fp32
                    nc.scalar.activation(
                        out_stage[:, m0 + 1 + j0:m0 + 1 + j0 + jn, :],
                        ps[:, 0:jn, :],
                        mybir.ActivationFunctionType.Relu,
                        bias=bg2_t[:, img:img + 1],
                        scale=gv_t[:, img:img + 1],
                    )
                    if m0 == -1 and j0 == 0:
                        continue
                    # store completed slot ranges: 8-slot chunks, then
                    # finer 4/2-slot chunks near the end for a shorter drain
                    hi = m0 + 1 + J
                    if hi <= 48 and hi % 8 == 0:
                        nc.gpsimd.dma_start(
                            out=out_ext[img, :, hi - 8:hi, :],
                            in_=out_stage[:, hi - 8:hi, :])
                    elif 48 < hi <= 62 and hi % 4 == 2:
                        nc.gpsimd.dma_start(
                            out=out_ext[img, :, hi - 4:hi, :],
                            in_=out_stage[:, hi - 4:hi, :])
                    elif hi > 62:
                        # slot 65 is a dead pad slot the host never reads
                        h1 = min(hi, A + 1)
                        eng = nc.gpsimd if hi == 64 else nc.sync
                        eng.dma_start(
                            out=out_ext[img, :, hi - 2:h1, :],
                            in_=out_stage[:, hi - 2:h1, :])


                # (chunked stores emitted inside the conv2 loop above)

    nc.compile()
    return nc


def _host_prep(x, gate_values, w1, b1, w2, b2):
    x = np.ascontiguousarray(np.asarray(x, dtype=np.float32))
    gate_values = np.asarray(gate_values, dtype=np.float32)
    w1 = np.asarray(w1, dtype=np.float32)
    b1 = np.asarray(b1, dtype=np.float32)
    w2 = np.asarray(w2, dtype=np.float32)
    b2 = np.asarray(b2, dtype=np.float32)

    g = gate_values * (gate_values > 0)                      # [B, C]
    wv1 = np.ascontiguousarray(_pack_weights_wrapped(w1).transpose(1, 0, 2)).astype(ml_dtypes.bfloat16)
    wv2 = np.ascontiguousarray(_pack_weights_wrapped(w2).transpose(1, 0, 2)).astype(ml_dtypes.bfloat16)

    in_maps = []
    for core in range(N_CORES):
        sl = slice(core * IMGS_PER_CORE, (core + 1) * IMGS_PER_CORE)
        gc = g[sl]                                           # [2, C]
        x_il = _interleave_x(x[sl])
        in_maps.append({
            "xil": x_il, "xw": _wrap_x(x[sl]),
            "wv1": wv1, "wv2": wv2,
            "gv": np.ascontiguousarray(np.tile(gc.T, (S, 1))),
            "bg1": np.ascontiguousarray(np.tile((gc * b1[None, :]).T, (S, 1))),
            "bg2": np.ascontiguousarray(np.tile((gc * b2[None, :]).T, (S, 1))),
        })
    return in_maps


_NC_CACHE = None


def _get_graph():
    global _NC_CACHE
    if _NC_CACHE is None:
        _NC_CACHE = _build_core_graph()
    return _NC_CACHE


def kernel(x, gate_values, w1, b1, w2, b2, _trace=False, **_ignored):
    from concourse.bass_utils import run_bass_kernel_spmd

    nc = _get_graph()
    in_maps = _host_prep(x, gate_values, w1, b1, w2, b2)
    res = run_bass_kernel_spmd(
        nc, in_maps, core_ids=list(range(N_CORES)), trace=_trace)
    outs = [_deinterleave_out(res.results[i]["out"]) for i in range(N_CORES)]
    full = np.concatenate(outs, axis=0).astype(np.float32)
    full += np.asarray(x, dtype=np.float32)
    if _trace:
        return full, res
    return full



# revision 3
# speedup vs baseline: 1.2043x; 1.2043x over previous
"""Trainium2 Bass kernel: gated MoE residual block (two 3x3 convs, C=32).

  g  = gate * (gate > 0)                          # [B, C]
  h  = relu((conv3x3(x, w1) + b1) * g)
  h2 = relu((conv3x3(h, w2) + b2) * g)
  out = h2 + x

Sharding: data-parallel over batch. 16 images -> 8 cores x 2 images.

Device algorithm (per core, per image) — 2x2 space-to-depth conv:
  - x arrives host-packed as x2[(2sr+sc)*32+ci, R', C'] = x[ci, 2R'-1+sr,
    2C'-1+sc] (bf16, odd-aligned 2x2 patches, zero halo baked in).
  - conv1 as 4 "alignment" matmuls per PSUM tile: K = 2x2 patch x 32ci =
    128, M = 2x2 output phase x 32co = 128, rhs slice offset (ar, ac).
    Output phases are even-aligned: psum[(2qr+qc)*32+co, (R, Cg)] =
    conv(x)[co, 2R+qr, 2Cg+qc].  9/16 weight-block density vs the 37.5%
    of a 4-row-interleave scheme -> 1.5x fewer PE cycles.
  - conv1 epilogue (ScalarE): h = relu(psum*g + b1*g) written into
    h[128, 130, 130] phase layout (halo borders pre-zeroed).  h's phase
    layout IS the even-aligned 2x2 patch layout conv2 needs: conv2 runs
    the same 4-alignment matmuls producing odd-aligned output phases
    out[(2or+oc)*32+co, (R, C)] = h2[co, 2R-1+or, 2C-1+oc], R in 0..128.
    No SBUF->SBUF re-layout DMA at all.
  - conv2 epilogue writes out_stage; chunked contiguous DMA to HBM;
    host de-interleaves and adds the +x residual in fp32.
"""

import numpy as np
import ml_dtypes

import concourse.bass as bass
import concourse.tile as tile
from concourse import bacc, mybir

B, C, H, W = 16, 32, 256, 256
IMGS_PER_CORE = 2
N_CORES = 8
F32 = mybir.dt.float32
BF16 = mybir.dt.bfloat16

G1 = H // 2          # 128 conv1 output groups per dim
GX = G1 + 1          # 129 x2 patch-grid size (odd-aligned, halo baked)
G2 = G1 + 1          # 129 conv2 output groups per dim (odd grid)
T1 = 4               # conv1 R-groups per PSUM tile (N = 4*128 = 512)
T2 = 3               # conv2 R-groups per PSUM tile (N = 3*129 = 387)
NT1 = G1 // T1       # 32 conv1 tiles
NT2 = G2 // T2       # 43 conv2 tiles
ALIGNS = ((0, 0), (0, 1), (1, 0), (1, 1))


def _pack_weights(w: np.ndarray) -> np.ndarray:
    """w: [C_out, C_in, 3, 3] (OIHW) -> [4, 128, 128] lhsT per alignment.

    wv[2ar+ac][(2sr+sc)*32+ci, (2qr+qc)*32+co] = w[co, ci, 2ar+sr-qr,
    2ac+sc-qc] when both taps lie in {0,1,2}; each (phase, tap) pair is
    covered by exactly one (alignment, patch-slot).
    """
    wv = np.zeros((4, 4 * C, 4 * C), dtype=np.float32)
    for ar, ac in ALIGNS:
        for sr in range(2):
            for sc in range(2):
                for qr in range(2):
                    for qc in range(2):
                        dy = 2 * ar + sr - qr
                        dx = 2 * ac + sc - qc
                        if 0 <= dy <= 2 and 0 <= dx <= 2:
                            kb = (2 * sr + sc) * C
                            mb = (2 * qr + qc) * C
                            wv[2 * ar + ac, kb:kb + C, mb:mb + C] = \
                                w[:, :, dy, dx].T
    return wv


def _pack_x2(x: np.ndarray) -> np.ndarray:
    """x: [n, C, H, W] f32 -> [n, 128, 129, 129] bf16 odd 2x2 patches."""
    n = x.shape[0]
    xb = x.astype(ml_dtypes.bfloat16)
    xp = np.zeros((n, C, H + 4, W + 4), dtype=ml_dtypes.bfloat16)
    xp[:, :, 1:1 + H, 1:1 + W] = xb
    v = xp[:, :, 0:2 * GX, 0:2 * GX].reshape(n, C, GX, 2, GX, 2)
    x2 = v.transpose(0, 3, 5, 1, 2, 4).reshape(n, 4 * C, GX, GX)
    return np.ascontiguousarray(x2)


def _unpack_out(dev: np.ndarray) -> np.ndarray:
    """dev: [n, 128, 129, 129] (odd phases) -> [n, C, H, W] f32."""
    v = np.asarray(dev).astype(np.float32).reshape(-1, 2, 2, C, G2, G2)
    big = v.transpose(0, 3, 4, 1, 5, 2).reshape(-1, C, 2 * G2, 2 * G2)
    return np.ascontiguousarray(big[:, :, 1:1 + H, 1:1 + W])


def _build_core_graph():
    nc = bacc.Bacc(None, target_bir_lowering=False, debug=False)

    x2_ext = nc.declare_dram_parameter("x2", [IMGS_PER_CORE, 4 * C, GX, GX], BF16, isOutput=False)
    wv1_ext = nc.declare_dram_parameter("wv1", [4 * C, 4, 4 * C], BF16, isOutput=False)
    wv2_ext = nc.declare_dram_parameter("wv2", [4 * C, 4, 4 * C], BF16, isOutput=False)
    gv_ext = nc.declare_dram_parameter("gv", [4 * C, IMGS_PER_CORE], F32, isOutput=False)
    bg1_ext = nc.declare_dram_parameter("bg1", [4 * C, IMGS_PER_CORE], F32, isOutput=False)
    bg2_ext = nc.declare_dram_parameter("bg2", [4 * C, IMGS_PER_CORE], F32, isOutput=False)
    out_ext = nc.declare_dram_parameter("out", [IMGS_PER_CORE, 4 * C, G2, G2], BF16, isOutput=True)

    RELU = mybir.ActivationFunctionType.Relu

    with tile.TileContext(nc) as tc:
        with (
            tc.tile_pool(name="const", bufs=1) as cpool,
            tc.tile_pool(name="xb", bufs=2) as xpool,
            tc.tile_pool(name="os", bufs=1) as ospool,
            tc.tile_pool(name="ps", bufs=8, space=bass.MemorySpace.PSUM) as pspool,
        ):
            wv1_t = cpool.tile([4 * C, 4, 4 * C], BF16)
            wv2_t = cpool.tile([4 * C, 4, 4 * C], BF16)
            gv_t = cpool.tile([4 * C, IMGS_PER_CORE], F32)
            bg1_t = cpool.tile([4 * C, IMGS_PER_CORE], F32)
            bg2_t = cpool.tile([4 * C, IMGS_PER_CORE], F32)
            h_t = cpool.tile([4 * C, GX + 1, GX + 1], BF16)

            # PE warm-up: dummy matmuls on zeroed scratch start the clock
            # ramp before the first real operands arrive (results unread)
            warm = cpool.tile([4 * C, 512], BF16, tag="warm")
            nc.vector.memset(warm[:], 0.0)
            wps = pspool.tile([4 * C, T1, G1], F32, tag="ps")
            for _ in range(6):
                nc.tensor.matmul(
                    wps[:], warm[:, 0:4 * C], warm[:],
                    start=True, stop=True, skip_group_check=True)

            # weights first (first-needed), from engines whose SEQ is idle
            nc.scalar.dma_start(out=wv1_t[:], in_=wv1_ext[:])
            nc.scalar.dma_start(out=wv2_t[:], in_=wv2_ext[:])
            # h halo: borders stay zero for the whole kernel
            nc.vector.memset(h_t[:, 0, :], 0.0)
            nc.vector.memset(h_t[:, GX, :], 0.0)
            nc.vector.memset(h_t[:, :, 0], 0.0)
            nc.vector.memset(h_t[:, :, GX], 0.0)

            # x2 chunk bounds, first-need order (conv1 tile t needs rows
            # <= 4t+5); first chunk small so tile 0 starts ASAP
            xc = [0, 5, 21, 37, 53, 69, 85, 101, 117, GX]
            # out store bounds (rows of out_stage, 43 tiles of 3 rows)
            oc_ = [0, 21, 42, 63, 84, 105, G2]

            for img in range(IMGS_PER_CORE):
                x2_t = xpool.tile([4 * C, GX, GX], BF16)
                out_t = ospool.tile([4 * C, G2, G2], BF16)

                for c0, c1 in zip(xc[:-1], xc[1:]):
                    nc.sync.dma_start(out=x2_t[:, c0:c1, :],
                                      in_=x2_ext[img, :, c0:c1, :])
                if img == 0:
                    # small consts are only needed at the first epilogue
                    nc.gpsimd.dma_start(out=gv_t[:], in_=gv_ext[:])
                    nc.gpsimd.dma_start(out=bg1_t[:], in_=bg1_ext[:])
                    nc.gpsimd.dma_start(out=bg2_t[:], in_=bg2_ext[:])

                # ---- conv1: x2 -> h (even phase layout, +1 halo offset) ----
                for t in range(NT1):
                    ps = pspool.tile([4 * C, T1, G1], F32, tag="ps")
                    for i, (ar, ac) in enumerate(ALIGNS):
                        nc.tensor.matmul(
                            ps[:], wv1_t[:, 2 * ar + ac, :],
                            x2_t[:, T1 * t + ar:T1 * t + ar + T1, ac:ac + G1],
                            start=(i == 0), stop=(i == 3),
                            skip_group_check=True)
                    nc.scalar.activation(
                        h_t[:, 1 + T1 * t:1 + T1 * t + T1, 1:1 + G1],
                        ps[:], RELU,
                        bias=bg1_t[:, img:img + 1],
                        scale=gv_t[:, img:img + 1])

                # ---- conv2: h -> out_t (odd phases) + chunked stores ----
                ostore = 0
                for t in range(NT2):
                    ps = pspool.tile([4 * C, T2, G2], F32, tag="ps")
                    for i, (ar, ac) in enumerate(ALIGNS):
                        nc.tensor.matmul(
                            ps[:], wv2_t[:, 2 * ar + ac, :],
                            h_t[:, T2 * t + ar:T2 * t + ar + T2, ac:ac + G2],
                            start=(i == 0), stop=(i == 3),
                            skip_group_check=True)
                    nc.scalar.activation(
                        out_t[:, T2 * t:T2 * t + T2, :], ps[:], RELU,
                        bias=bg2_t[:, img:img + 1],
                        scale=gv_t[:, img:img + 1])
                    hi = T2 * t + T2
                    if hi >= oc_[ostore + 1]:
                        a, b = oc_[ostore], oc_[ostore + 1]
                        nc.gpsimd.dma_start(out=out_ext[img, :, a:b, :],
                                            in_=out_t[:, a:b, :])
                        ostore += 1

    nc.compile()
    return nc


def _host_prep(x, gate_values, w1, b1, w2, b2):
    x = np.ascontiguousarray(np.asarray(x, dtype=np.float32))
    gate_values = np.asarray(gate_values, dtype=np.float32)
    w1 = np.asarray(w1, dtype=np.float32)
    b1 = np.asarray(b1, dtype=np.float32)
    w2 = np.asarray(w2, dtype=np.float32)
    b2 = np.asarray(b2, dtype=np.float32)

    g = gate_values * (gate_values > 0)                      # [B, C]
    wv1 = np.ascontiguousarray(
        _pack_weights(w1).transpose(1, 0, 2)).astype(ml_dtypes.bfloat16)
    wv2 = np.ascontiguousarray(
        _pack_weights(w2).transpose(1, 0, 2)).astype(ml_dtypes.bfloat16)

    in_maps = []
    for core in range(N_CORES):
        sl = slice(core * IMGS_PER_CORE, (core + 1) * IMGS_PER_CORE)
        gc = g[sl]                                           # [2, C]
        in_maps.append({
            "x2": _pack_x2(x[sl]),
            "wv1": wv1, "wv2": wv2,
            "gv": np.ascontiguousarray(np.tile(gc.T, (4, 1))),
            "bg1": np.ascontiguousarray(np.tile((gc * b1[None, :]).T, (4, 1))),
            "bg2": np.ascontiguousarray(np.tile((gc * b2[None, :]).T, (4, 1))),
        })
    return in_maps


_NC_CACHE = None


def _get_graph():
    global _NC_CACHE
    if _NC_CACHE is None:
        _NC_CACHE = _build_core_graph()
    return _NC_CACHE


def kernel(x, gate_values, w1, b1, w2, b2, _trace=False, **_ignored):
    from concourse.bass_utils import run_bass_kernel_spmd

    nc = _get_graph()
    in_maps = _host_prep(x, gate_values, w1, b1, w2, b2)
    res = run_bass_kernel_spmd(
        nc, in_maps, core_ids=list(range(N_CORES)), trace=_trace)
    outs = [_unpack_out(res.results[i]["out"]) for i in range(N_CORES)]
    full = np.concatenate(outs, axis=0).astype(np.float32)
    full += np.asarray(x, dtype=np.float32)
    if _trace:
        return full, res
    return full


# revision 5
# speedup vs baseline: 2.7255x; 2.2631x over previous
"""Trainium2 Bass kernel: gated MoE residual block (two 3x3 convs, C=32).

  g  = gate * (gate > 0)                          # [B, C]
  h  = relu((conv3x3(x, w1) + b1) * g)
  h2 = relu((conv3x3(h, w2) + b2) * g)
  out = h2 + x

Sharding: data-parallel over batch. 16 images -> 8 cores x 2 images.

Device algorithm (per core, per image) — 2x2 space-to-depth conv in fp8
with DoubleRow matmuls:
  - x host-packed as x2[(2sr+sc)*32+ci, R', C'] = x[ci, 2R'-1+sr, 2C'-1+sc]
    (fp8 e4m3, odd-aligned 2x2 patches, zero halo baked in).
  - gating g and a fixed power-of-2 scale S=16 are folded into per-image
    fp8 weights (wv = fp8(S*g*w)), so every epilogue is a pure
    relu(psum + bias): one instruction on ANY of ScalarE / VectorE /
    PoolE -> the epilogue load is split across all three engines.
  - conv as 2 DoubleRow matmuls per PSUM tile: the 2 k-tiles are the two
    row-alignments (ar), the 2 matmuls the column-alignments (ac).
    K = 2x(2x2 patch x 32ci), M = 2x2 phase x 32co.  rhs uses a flat
    overlapped AP [128, 2 (stride row), N (stride 1)] that runs across
    row boundaries: the junk lands only in a dead PSUM pad column.
    fp8 DoubleRow = 0.5 cycles/row -> 4x the bf16 matmul throughput.
  - h (= S * true h, fp8) is written in phase layout with halo; conv2
    reads it directly with even-aligned patches (odd output phases).
  - out (= S^2 * true h2, bf16) staged and chunk-DMA'd; host
    de-interleaves, divides by S^2, adds the +x residual in fp32.
"""

import numpy as np
import ml_dtypes

import bass_rust
import concourse.bass as bass
import concourse.tile as tile
from concourse import bacc, mybir

B, C, H, W = 16, 32, 256, 256
IMGS_PER_CORE = 2
N_CORES = 8
F32 = mybir.dt.float32
BF16 = mybir.dt.bfloat16
FP8 = mybir.dt.float8e4
NP_FP8 = ml_dtypes.float8_e4m3

S = 16.0             # fp8 weight scale (power of 2; h stored as S*h)
G1 = H // 2          # 128 conv1 output groups per dim
GX = G1 + 1          # 129 x2 patch-grid size (odd-aligned, halo baked)
G2 = G1 + 1          # 129 conv2 output groups per dim (odd grid)
T = 3                # R-groups per PSUM tile
NT1 = 43             # conv1 tiles (42 full + one T=2)
NT2 = 43             # conv2 tiles (43 x 3 = 129)
HW_ROW = GX + 1      # h_t row length 130 (left/right halo cols)

DR = mybir.MatmulPerfMode.DoubleRow
ADD = None  # set lazily (mybir.AluOpType)


def _pack_weights(w: np.ndarray) -> np.ndarray:
    """w: [C_out, C_in, 3, 3] (OIHW) -> [2, 2, 128, 128] lhsT[ar][ac].

    wv[ar, ac][(2sr+sc)*32+ci, (2qr+qc)*32+co] = w[co, ci, 2ar+sr-qr,
    2ac+sc-qc] when both taps lie in {0,1,2}.
    """
    wv = np.zeros((2, 2, 4 * C, 4 * C), dtype=np.float32)
    for ar in range(2):
        for ac in range(2):
            for sr in range(2):
                for sc in range(2):
                    for qr in range(2):
                        for qc in range(2):
                            dy = 2 * ar + sr - qr
                            dx = 2 * ac + sc - qc
                            if 0 <= dy <= 2 and 0 <= dx <= 2:
                                kb = (2 * sr + sc) * C
                                mb = (2 * qr + qc) * C
                                wv[ar, ac, kb:kb + C, mb:mb + C] = \
                                    w[:, :, dy, dx].T
    return wv


def _pack_x2(x: np.ndarray) -> np.ndarray:
    """x: [n, C, H, W] f32 -> [n, 128, 129, 129] fp8 odd 2x2 patches."""
    n = x.shape[0]
    xp = np.zeros((n, C, H + 4, W + 4), dtype=np.float32)
    xp[:, :, 1:1 + H, 1:1 + W] = x
    v = xp[:, :, 0:2 * GX, 0:2 * GX].reshape(n, C, GX, 2, GX, 2)
    x2 = v.transpose(0, 3, 5, 1, 2, 4).reshape(n, 4 * C, GX, GX)
    return np.ascontiguousarray(x2.astype(NP_FP8))


def _unpack_out(dev: np.ndarray) -> np.ndarray:
    """dev: [n, 128, 129, 129] (odd phases, S^2-scaled) -> [n,C,H,W] f32."""
    v = np.asarray(dev).astype(np.float32).reshape(-1, 2, 2, C, G2, G2)
    big = v.transpose(0, 3, 4, 1, 5, 2).reshape(-1, C, 2 * G2, 2 * G2)
    return np.ascontiguousarray(big[:, :, 1:1 + H, 1:1 + W]) * (1.0 / (S * S))


def _ap_ktile(t_ap, r0: int, c0: int, row_w: int, n: int):
    """Overlapped rhs AP [128, 2 (stride row_w), n (stride 1)] into a 3D
    tile at (row r0, col c0): the DoubleRow k-tiles are rows r0, r0+1."""
    base = t_ap[:, r0:r0 + 1, c0:c0 + 1]
    a = base.copy()
    p = base.ap[0]
    a.ap = bass_rust.VecI64Pair([[p[0], p[1]], [row_w, 2], [1, n]])
    return a


def _ap_flat(t_ap, n: int):
    """Flat [128, n] AP over a 3D PSUM tile's first n elements."""
    base = t_ap[:, 0:1, 0:1]
    a = base.copy()
    p = base.ap[0]
    a.ap = bass_rust.VecI64Pair([[p[0], p[1]], [1, n]])
    return a


# epilogue engine split: deficit-weighted round robin by engine rate
_EPI_RATES = (1.0 / 463.0, 1.0 / 528.0, 1.0 / 632.0)   # Act, DVE, Pool


def _mk_assign(n: int):
    w = [r / sum(_EPI_RATES) for r in _EPI_RATES]
    credit = [0.0, 0.0, 0.0]
    out = []
    for _ in range(n):
        for j in range(3):
            credit[j] += w[j]
        i = max(range(3), key=lambda j: credit[j])
        credit[i] -= 1.0
        out.append(i)
    return out


def _build_core_graph():
    nc = bacc.Bacc(None, target_bir_lowering=False, debug=False)

    x2_ext = nc.declare_dram_parameter("x2", [IMGS_PER_CORE, 4 * C, GX, GX], FP8, isOutput=False)
    wv1_ext = nc.declare_dram_parameter("wv1", [4 * C, IMGS_PER_CORE, 2, 2, 4 * C], FP8, isOutput=False)
    wv2_ext = nc.declare_dram_parameter("wv2", [4 * C, IMGS_PER_CORE, 2, 2, 4 * C], FP8, isOutput=False)
    bg1_ext = nc.declare_dram_parameter("bg1", [4 * C, IMGS_PER_CORE], F32, isOutput=False)
    bg2_ext = nc.declare_dram_parameter("bg2", [4 * C, IMGS_PER_CORE], F32, isOutput=False)
    out_ext = nc.declare_dram_parameter("out", [IMGS_PER_CORE, 4 * C, G2, G2], BF16, isOutput=True)

    RELU = mybir.ActivationFunctionType.Relu
    A_ADD = mybir.AluOpType.add
    A_MAX = mybir.AluOpType.max
    assign = _mk_assign(NT1 + NT2)

    with tile.TileContext(nc) as tc:
        with (
            tc.tile_pool(name="const", bufs=1) as cpool,
            tc.tile_pool(name="xb", bufs=2) as xpool,
            tc.tile_pool(name="os", bufs=1) as ospool,
            tc.tile_pool(name="ps", bufs=8, space=bass.MemorySpace.PSUM) as pspool,
        ):
            wv1_t = cpool.tile([4 * C, IMGS_PER_CORE, 2, 2, 4 * C], FP8)
            wv2_t = cpool.tile([4 * C, IMGS_PER_CORE, 2, 2, 4 * C], FP8)
            bg1_t = cpool.tile([4 * C, IMGS_PER_CORE], F32)
            bg2_t = cpool.tile([4 * C, IMGS_PER_CORE], F32)
            # h rows 0 and 129 are the top/bottom halo; row 130 is scratch
            # absorbing the 1-element flat-AP overrun of the last tile
            h_t = cpool.tile([4 * C, HW_ROW + 1, HW_ROW], FP8)

            # PE warm-up: dummy matmuls start the clock ramp while DMAs
            # stream in; an early tiny activation pre-loads the Relu table
            warm = cpool.tile([4 * C, 512], BF16, tag="warm")
            warm8 = cpool.tile([4 * C, 16], BF16, tag="warm8")
            nc.vector.memset(warm[:], 0.0)
            nc.vector.memset(warm8[:], 0.0)
            wps = pspool.tile([4 * C, 512], F32, tag="ps")
            for _ in range(8):
                nc.tensor.matmul(
                    wps[:], warm[:, 0:4 * C], warm[:],
                    start=True, stop=True, skip_group_check=True)
            nc.scalar.activation(warm8[:], warm8[:], RELU)

            # weights first (first-needed), from the otherwise-idle Act SEQ
            nc.scalar.dma_start(out=wv1_t[:], in_=wv1_ext[:])
            nc.scalar.dma_start(out=wv2_t[:], in_=wv2_ext[:])
            # h halo borders + scratch row stay zero the whole kernel
            nc.vector.memset(h_t[:, 0, :], 0.0)
            nc.vector.memset(h_t[:, GX, :], 0.0)
            nc.vector.memset(h_t[:, HW_ROW, :], 0.0)
            nc.vector.memset(h_t[:, :, 0], 0.0)
            nc.vector.memset(h_t[:, :, HW_ROW - 1], 0.0)

            # x2 chunk bounds, first-need order (conv1 tile t needs rows
            # <= 3t+3); first chunks small so tile 0 starts ASAP
            xc = [0, 4, 10, 18, 34, 50, 66, 82, 98, 114, GX]
            # out store bounds (rows of out_t, after tiles 6,13,...,42)
            oc_ = [0, 21, 42, 63, 84, 105, G2]

            def epilogue(eng_i, out_ap, in_ap, bg_ap):
                if eng_i == 0:
                    nc.scalar.activation(out_ap, in_ap, RELU, bias=bg_ap)
                elif eng_i == 1:
                    nc.vector.tensor_scalar(
                        out_ap, in_ap, bg_ap, 0.0, A_ADD, A_MAX)
                else:
                    nc.gpsimd.tensor_scalar(
                        out_ap, in_ap, bg_ap, 0.0, A_ADD, A_MAX)

            for img in range(IMGS_PER_CORE):
                # row 129 is scratch for the flat-AP overrun
                x2_t = xpool.tile([4 * C, GX + 1, GX], FP8)
                out_t = ospool.tile([4 * C, G2, G2], BF16)
                nc.vector.memset(x2_t[:, GX, :], 0.0)

                for c0, c1 in zip(xc[:-1], xc[1:]):
                    nc.sync.dma_start(out=x2_t[:, c0:c1, :],
                                      in_=x2_ext[img, :, c0:c1, :])
                if img == 0:
                    nc.gpsimd.dma_start(out=bg1_t[:], in_=bg1_ext[:])
                    nc.gpsimd.dma_start(out=bg2_t[:], in_=bg2_ext[:])

                # ---- conv1: x2 -> h (even phases, +1 halo offset) ----
                for t in range(NT1):
                    tt = 3 * t
                    rows = T if t < NT1 - 1 else G1 - 3 * (NT1 - 1)
                    n = rows * GX
                    ps = pspool.tile([4 * C, T, GX], F32, tag="ps")
                    out_flat = _ap_flat(ps, n)
                    for i, ac in enumerate((0, 1)):
                        nc.tensor.matmul(
                            out_flat,
                            wv1_t[:, img, ac, :, :],
                            _ap_ktile(x2_t, tt, ac, GX, n),
                            start=(i == 0), stop=(i == 1),
                            perf_mode=DR, skip_group_check=True)
                    epilogue(assign[t],
                             h_t[:, 1 + tt:1 + tt + rows, 1:1 + G1],
                             ps[:, 0:rows, 0:G1],
                             bg1_t[:, img:img + 1])

                # ---- conv2: h -> out_t (odd phases) + chunked stores ----
                ostore = 0
                for t in range(NT2):
                    tt = 3 * t
                    n = T * HW_ROW
                    ps = pspool.tile([4 * C, T, HW_ROW], F32, tag="ps")
                    out_flat = _ap_flat(ps, n)
                    for i, ac in enumerate((0, 1)):
                        nc.tensor.matmul(
                            out_flat,
                            wv2_t[:, img, ac, :, :],
                            _ap_ktile(h_t, tt, ac, HW_ROW, n),
                            start=(i == 0), stop=(i == 1),
                            perf_mode=DR, skip_group_check=True)
                    epilogue(assign[NT1 + t],
                             out_t[:, tt:tt + T, :],
                             ps[:, 0:T, 0:G2],
                             bg2_t[:, img:img + 1])
                    hi = tt + T
                    if hi >= oc_[ostore + 1]:
                        a, b = oc_[ostore], oc_[ostore + 1]
                        nc.gpsimd.dma_start(out=out_ext[img, :, a:b, :],
                                            in_=out_t[:, a:b, :])
                        ostore += 1

    nc.compile()
    return nc


def _host_prep(x, gate_values, w1, b1, w2, b2):
    x = np.ascontiguousarray(np.asarray(x, dtype=np.float32))
    gate_values = np.asarray(gate_values, dtype=np.float32)
    w1 = np.asarray(w1, dtype=np.float32)
    b1 = np.asarray(b1, dtype=np.float32)
    w2 = np.asarray(w2, dtype=np.float32)
    b2 = np.asarray(b2, dtype=np.float32)

    g = gate_values * (gate_values > 0)                      # [B, C]

    in_maps = []
    for core in range(N_CORES):
        sl = slice(core * IMGS_PER_CORE, (core + 1) * IMGS_PER_CORE)
        gc = g[sl]                                           # [2, C]
        wv1 = np.zeros((4 * C, IMGS_PER_CORE, 2, 2, 4 * C), dtype=NP_FP8)
        wv2 = np.zeros_like(wv1)
        for img in range(IMGS_PER_CORE):
            p1 = _pack_weights(S * gc[img][:, None, None, None] * w1)
            p2 = _pack_weights(S * gc[img][:, None, None, None] * w2)
            for ac in range(2):
                for ar in range(2):
                    wv1[:, img, ac, ar, :] = p1[ar, ac].astype(NP_FP8)
                    wv2[:, img, ac, ar, :] = p2[ar, ac].astype(NP_FP8)
        in_maps.append({
            "x2": _pack_x2(x[sl]),
            "wv1": np.ascontiguousarray(wv1),
            "wv2": np.ascontiguousarray(wv2),
            "bg1": np.ascontiguousarray(np.tile(S * (gc * b1[None, :]).T, (4, 1))),
            "bg2": np.ascontiguousarray(np.tile(S * S * (gc * b2[None, :]).T, (4, 1))),
        })
    return in_maps


_NC_CACHE = None


def _get_graph():
    global _NC_CACHE
    if _NC_CACHE is None:
        _NC_CACHE = _build_core_graph()
    return _NC_CACHE


def kernel(x, gate_values, w1, b1, w2, b2, _trace=False, **_ignored):
    from concourse.bass_utils import run_bass_kernel_spmd

    nc = _get_graph()
    in_maps = _host_prep(x, gate_values, w1, b1, w2, b2)
    res = run_bass_kernel_spmd(
        nc, in_maps, core_ids=list(range(N_CORES)), trace=_trace)
    outs = [_unpack_out(res.results[i]["out"]) for i in range(N_CORES)]
    full = np.concatenate(outs, axis=0).astype(np.float32)
    full += np.asarray(x, dtype=np.float32)
    if _trace:
        return full, res
    return full


# revision 6
# speedup vs baseline: 3.0813x; 1.1306x over previous
"""Trainium2 Bass kernel: gated MoE residual block (two 3x3 convs, C=32).

  g  = gate * (gate > 0)                          # [B, C]
  h  = relu((conv3x3(x, w1) + b1) * g)
  h2 = relu((conv3x3(h, w2) + b2) * g)
  out = h2 + x

Sharding: data-parallel over batch. 16 images -> 8 cores x 2 images.

Device algorithm (per core, per image) — 2x2 space-to-depth conv in fp8
with DoubleRow matmuls:
  - x host-packed as x2[(2sr+sc)*32+ci, R', C'] = x[ci, 2R'-1+sr, 2C'-1+sc]
    (fp8 e4m3, odd-aligned 2x2 patches, zero halo baked in).
  - gating g and a fixed power-of-2 scale S=16 are folded into per-image
    fp8 weights (wv = fp8(S*g*w)), so every epilogue is a pure
    relu(psum + bias) -> one instruction on ScalarE or VectorE (GPSIMD
    cannot read PSUM on TRN2, so Pool only issues the output DMAs).
  - conv as 2 DoubleRow matmuls per PSUM group: the 2 k-tiles are the two
    row-alignments (ar), the 2 matmuls the column-alignments (ac).
    K = 2x(2x2 patch x 32ci), M = 2x2 phase x 32co.  rhs uses a flat
    overlapped AP [128, 2 (stride row), N (stride 1)] that runs across
    row boundaries: the junk lands only in a dead PSUM pad column.
    fp8 DoubleRow = 0.5 cycles/row -> 4x the bf16 matmul throughput.
  - PSUM slots hold TWO 3-row groups (2 banks, one group per bank); one
    fused epilogue instruction drains both, halving per-instr overheads.
  - h (= S * true h, fp8) is written in phase layout with halo; conv2
    reads it directly with even-aligned patches (odd output phases).
  - out (= S^2 * true h2, fp8; max ~204 < 240) staged and chunk-DMA'd;
    host de-interleaves, divides by S^2, adds the +x residual in fp32.
"""

import numpy as np
import ml_dtypes

import bass_rust
import concourse.bass as bass
import concourse.tile as tile
from concourse import bacc, mybir

B, C, H, W = 16, 32, 256, 256
IMGS_PER_CORE = 2
N_CORES = 8
F32 = mybir.dt.float32
BF16 = mybir.dt.bfloat16
FP8 = mybir.dt.float8e4
NP_FP8 = ml_dtypes.float8_e4m3

S = 16.0             # fp8 weight scale (power of 2; h stored as S*h)
G1 = H // 2          # 128 conv1 output groups per dim
GX = G1 + 1          # 129 x2 patch-grid size (odd-aligned, halo baked)
G2 = G1 + 1          # 129 conv2 output groups per dim (odd grid)
HW_ROW = GX + 1      # h_t row length 130 (left/right halo cols)
NP1 = 21             # conv1 pairs (2x3 rows each) + one 2-row single
NP2 = 21             # conv2 pairs + one 3-row single

DR = mybir.MatmulPerfMode.DoubleRow


def _pack_weights(w: np.ndarray) -> np.ndarray:
    """w: [C_out, C_in, 3, 3] (OIHW) -> [2, 2, 128, 128] lhsT[ar][ac].

    wv[ar, ac][(2sr+sc)*32+ci, (2qr+qc)*32+co] = w[co, ci, 2ar+sr-qr,
    2ac+sc-qc] when both taps lie in {0,1,2}.
    """
    wv = np.zeros((2, 2, 4 * C, 4 * C), dtype=np.float32)
    for ar in range(2):
        for ac in range(2):
            for sr in range(2):
                for sc in range(2):
                    for qr in range(2):
                        for qc in range(2):
                            dy = 2 * ar + sr - qr
                            dx = 2 * ac + sc - qc
                            if 0 <= dy <= 2 and 0 <= dx <= 2:
                                kb = (2 * sr + sc) * C
                                mb = (2 * qr + qc) * C
                                wv[ar, ac, kb:kb + C, mb:mb + C] = \
                                    w[:, :, dy, dx].T
    return wv


def _pack_x2(x: np.ndarray) -> np.ndarray:
    """x: [n, C, H, W] f32 -> [n, 128, 129, 129] fp8 odd 2x2 patches."""
    n = x.shape[0]
    xp = np.zeros((n, C, H + 4, W + 4), dtype=np.float32)
    xp[:, :, 1:1 + H, 1:1 + W] = x
    v = xp[:, :, 0:2 * GX, 0:2 * GX].reshape(n, C, GX, 2, GX, 2)
    x2 = v.transpose(0, 3, 5, 1, 2, 4).reshape(n, 4 * C, GX, GX)
    return np.ascontiguousarray(x2.astype(NP_FP8))


def _unpack_out(dev: np.ndarray) -> np.ndarray:
    """dev: [n, 128, 129, 129] (odd phases, S^2-scaled) -> [n,C,H,W] f32."""
    v = np.asarray(dev).astype(np.float32).reshape(-1, 2, 2, C, G2, G2)
    big = v.transpose(0, 3, 4, 1, 5, 2).reshape(-1, C, 2 * G2, 2 * G2)
    return np.ascontiguousarray(big[:, :, 1:1 + H, 1:1 + W]) * (1.0 / (S * S))


def _mk_ap(base, dims):
    """Custom AP with `base`'s tensor/offset/partition dim and free `dims`
    = [[stride, num], ...]."""
    a = base.copy()
    p = base.ap[0]
    a.ap = bass_rust.VecI64Pair([[p[0], p[1]]] + dims)
    return a


def _build_core_graph():
    nc = bacc.Bacc(None, target_bir_lowering=False, debug=False)

    x2_ext = nc.declare_dram_parameter("x2", [IMGS_PER_CORE, 4 * C, GX, GX], FP8, isOutput=False)
    wv1_ext = nc.declare_dram_parameter("wv1", [4 * C, IMGS_PER_CORE, 2, 2, 4 * C], FP8, isOutput=False)
    wv2_ext = nc.declare_dram_parameter("wv2", [4 * C, IMGS_PER_CORE, 2, 2, 4 * C], FP8, isOutput=False)
    bg1_ext = nc.declare_dram_parameter("bg1", [4 * C, IMGS_PER_CORE], F32, isOutput=False)
    bg2_ext = nc.declare_dram_parameter("bg2", [4 * C, IMGS_PER_CORE], F32, isOutput=False)
    out_ext = nc.declare_dram_parameter("out", [IMGS_PER_CORE, 4 * C, G2, G2], FP8, isOutput=True)

    RELU = mybir.ActivationFunctionType.Relu
    A_ADD = mybir.AluOpType.add
    A_MAX = mybir.AluOpType.max

    with tile.TileContext(nc) as tc:
        with (
            tc.tile_pool(name="const", bufs=1) as cpool,
            tc.tile_pool(name="xb", bufs=2) as xpool,
            tc.tile_pool(name="os", bufs=1) as ospool,
            tc.tile_pool(name="ps", bufs=4, space=bass.MemorySpace.PSUM) as pspool,
        ):
            wv1_t = cpool.tile([4 * C, IMGS_PER_CORE, 2, 2, 4 * C], FP8)
            wv2_t = cpool.tile([4 * C, IMGS_PER_CORE, 2, 2, 4 * C], FP8)
            bg1_t = cpool.tile([4 * C, IMGS_PER_CORE], F32)
            bg2_t = cpool.tile([4 * C, IMGS_PER_CORE], F32)
            # h rows 0 and 129 are the top/bottom halo; row 130 is scratch
            # absorbing the 1-element flat-AP overrun of the last tile
            h_t = cpool.tile([4 * C, HW_ROW + 1, HW_ROW], FP8)

            # PE warm-up: dummy matmuls start the clock ramp while DMAs
            # stream in; an early tiny activation pre-loads the Relu table
            warm = cpool.tile([4 * C, 512], BF16, tag="warm")
            warm8 = cpool.tile([4 * C, 16], BF16, tag="warm8")
            nc.vector.memset(warm[:], 0.0)
            nc.vector.memset(warm8[:], 0.0)
            wps = pspool.tile([4 * C, 2, 512], F32, tag="ps")
            for _ in range(8):
                nc.tensor.matmul(
                    wps[:, 0, :], warm[:, 0:4 * C], warm[:],
                    start=True, stop=True, skip_group_check=True)
            nc.scalar.activation(warm8[:], warm8[:], RELU)

            # weights first (first-needed), from the otherwise-idle Act SEQ
            nc.scalar.dma_start(out=wv1_t[:], in_=wv1_ext[:])
            nc.scalar.dma_start(out=wv2_t[:], in_=wv2_ext[:])
            # h halo borders + scratch row stay zero the whole kernel
            nc.vector.memset(h_t[:, 0, :], 0.0)
            nc.vector.memset(h_t[:, GX, :], 0.0)
            nc.vector.memset(h_t[:, HW_ROW, :], 0.0)
            nc.vector.memset(h_t[:, :, 0], 0.0)
            nc.vector.memset(h_t[:, :, HW_ROW - 1], 0.0)

            # x2 chunk bounds, first-need order (conv1 pair p needs rows
            # <= 6p+6); first chunks small so pair 0 starts ASAP
            xc = [0, 7, 19, 35, 51, 67, 83, 99, 115, GX]
            # out store bounds (rows of out_t), flushed after pair p ends
            oc_ = [0, 24, 48, 72, 96, 126, G2]

            def conv_group(wv_t, img, src, r0, row_w, n, ps, j):
                """One 3-row output group: 2 DoubleRow matmuls into bank j."""
                out_flat = _mk_ap(ps[:, j:j + 1, 0:1], [[1, n]])
                for i, ac in enumerate((0, 1)):
                    rhs = _mk_ap(src[:, r0:r0 + 1, ac:ac + 1],
                                 [[row_w, 2], [1, n]])
                    nc.tensor.matmul(
                        out_flat, wv_t[:, img, ac, :, :], rhs,
                        start=(i == 0), stop=(i == 1),
                        perf_mode=DR, skip_group_check=True)

            def epilogue(eng_i, out_ap, in_ap, bg_ap):
                if eng_i == 0:
                    nc.scalar.activation(out_ap, in_ap, RELU, bias=bg_ap)
                else:
                    nc.vector.tensor_scalar(
                        out_ap, in_ap, bg_ap, 0.0, A_ADD, A_MAX)

            # deficit-weighted Act/DVE interleave (Act is ~18% faster)
            def mk_assign(n, wa=1 / 788.0, wd=1 / 931.0):
                credit = [0.0, 0.0]
                out = []
                for _ in range(n):
                    credit[0] += wa / (wa + wd)
                    credit[1] += wd / (wa + wd)
                    i = 0 if credit[0] >= credit[1] else 1
                    credit[i] -= 1.0
                    out.append(i)
                return out

            assign = mk_assign((NP1 + NP2 + 2) * IMGS_PER_CORE)
            ai = 0

            for img in range(IMGS_PER_CORE):
                # row 129 is scratch for the flat-AP overrun
                x2_t = xpool.tile([4 * C, GX + 1, GX], FP8)
                out_t = ospool.tile([4 * C, G2, G2], FP8)
                nc.vector.memset(x2_t[:, GX, :], 0.0)

                for c0, c1 in zip(xc[:-1], xc[1:]):
                    nc.sync.dma_start(out=x2_t[:, c0:c1, :],
                                      in_=x2_ext[img, :, c0:c1, :])
                if img == 0:
                    nc.gpsimd.dma_start(out=bg1_t[:], in_=bg1_ext[:])
                    nc.gpsimd.dma_start(out=bg2_t[:], in_=bg2_ext[:])

                # ---- conv1: x2 -> h (even phases, +1 halo offset) ----
                for p in range(NP1 + 1):
                    ps = pspool.tile([4 * C, 2, 512], F32, tag="ps")
                    if p < NP1:
                        for j in range(2):
                            conv_group(wv1_t, img, x2_t, 6 * p + 3 * j,
                                       GX, 3 * GX, ps, j)
                        epilogue(assign[ai],
                                 h_t[:, 1 + 6 * p:7 + 6 * p, 1:1 + G1],
                                 _mk_ap(ps[:, 0:1, 0:1],
                                        [[512, 2], [GX, 3], [1, G1]]),
                                 bg1_t[:, img:img + 1])
                    else:
                        # rows 126..127: one 2-row group
                        conv_group(wv1_t, img, x2_t, 126, GX, 2 * GX, ps, 0)
                        epilogue(assign[ai],
                                 h_t[:, 127:129, 1:1 + G1],
                                 _mk_ap(ps[:, 0:1, 0:1], [[GX, 2], [1, G1]]),
                                 bg1_t[:, img:img + 1])
                    ai += 1

                # ---- conv2: h -> out_t (odd phases) + chunked stores ----
                ostore = 0
                for p in range(NP2 + 1):
                    ps = pspool.tile([4 * C, 2, 512], F32, tag="ps")
                    if p < NP2:
                        for j in range(2):
                            conv_group(wv2_t, img, h_t, 6 * p + 3 * j,
                                       HW_ROW, 3 * HW_ROW, ps, j)
                        epilogue(assign[ai],
                                 out_t[:, 6 * p:6 * p + 6, :],
                                 _mk_ap(ps[:, 0:1, 0:1],
                                        [[512, 2], [HW_ROW, 3], [1, G2]]),
                                 bg2_t[:, img:img + 1])
                        hi = 6 * p + 6
                    else:
                        # rows 126..128: one 3-row group
                        conv_group(wv2_t, img, h_t, 126, HW_ROW,
                                   3 * HW_ROW, ps, 0)
                        epilogue(assign[ai],
                                 out_t[:, 126:129, :],
                                 _mk_ap(ps[:, 0:1, 0:1],
                                        [[HW_ROW, 3], [1, G2]]),
                                 bg2_t[:, img:img + 1])
                        hi = G2
                    ai += 1
                    if hi >= oc_[ostore + 1]:
                        a, b = oc_[ostore], oc_[ostore + 1]
                        nc.gpsimd.dma_start(out=out_ext[img, :, a:b, :],
                                            in_=out_t[:, a:b, :])
                        ostore += 1

    nc.compile()
    return nc


def _host_prep(x, gate_values, w1, b1, w2, b2):
    x = np.ascontiguousarray(np.asarray(x, dtype=np.float32))
    gate_values = np.asarray(gate_values, dtype=np.float32)
    w1 = np.asarray(w1, dtype=np.float32)
    b1 = np.asarray(b1, dtype=np.float32)
    w2 = np.asarray(w2, dtype=np.float32)
    b2 = np.asarray(b2, dtype=np.float32)

    g = gate_values * (gate_values > 0)                      # [B, C]

    in_maps = []
    for core in range(N_CORES):
        sl = slice(core * IMGS_PER_CORE, (core + 1) * IMGS_PER_CORE)
        gc = g[sl]                                           # [2, C]
        wv1 = np.zeros((4 * C, IMGS_PER_CORE, 2, 2, 4 * C), dtype=NP_FP8)
        wv2 = np.zeros_like(wv1)
        for img in range(IMGS_PER_CORE):
            p1 = _pack_weights(S * gc[img][:, None, None, None] * w1)
            p2 = _pack_weights(S * gc[img][:, None, None, None] * w2)
            for ac in range(2):
                for ar in range(2):
                    wv1[:, img, ac, ar, :] = p1[ar, ac].astype(NP_FP8)
                    wv2[:, img, ac, ar, :] = p2[ar, ac].astype(NP_FP8)
        in_maps.append({
            "x2": _pack_x2(x[sl]),
            "wv1": np.ascontiguousarray(wv1),
            "wv2": np.ascontiguousarray(wv2),
            "bg1": np.ascontiguousarray(np.tile(S * (gc * b1[None, :]).T, (4, 1))),
            "bg2": np.ascontiguousarray(np.tile(S * S * (gc * b2[None, :]).T, (4, 1))),
        })
    return in_maps


_NC_CACHE = None


def _get_graph():
    global _NC_CACHE
    if _NC_CACHE is None:
        _NC_CACHE = _build_core_graph()
    return _NC_CACHE


def kernel(x, gate_values, w1, b1, w2, b2, _trace=False, **_ignored):
    from concourse.bass_utils import run_bass_kernel_spmd

    nc = _get_graph()
    in_maps = _host_prep(x, gate_values, w1, b1, w2, b2)
    res = run_bass_kernel_spmd(
        nc, in_maps, core_ids=list(range(N_CORES)), trace=_trace)
    outs = [_unpack_out(res.results[i]["out"]) for i in range(N_CORES)]
    full = np.concatenate(outs, axis=0).astype(np.float32)
    full += np.asarray(x, dtype=np.float32)
    if _trace:
        return full, res
    return full
